# revision 18
# baseline (speedup 1.0000x reference)
import os
import sys

import ml_dtypes
import numpy as np

for p in ("/opt/trn_rl_repo",):
    if p not in sys.path:
        sys.path.insert(0, p)

import concourse.bass as bass  # noqa: E402
import concourse.tile as tile  # noqa: E402
from concourse import bacc, mybir  # noqa: E402
from concourse.bass_utils import run_bass_kernel_spmd  # noqa: E402

B, N, D = 128, 512, 512
NCORES = 8
BPC = B // NCORES  # 16 batch items per core
F32 = mybir.dt.float32
BF16 = mybir.dt.bfloat16

LAST_RESULTS = None


def _hadamard(n: int) -> np.ndarray:
    H = np.array([[1.0]], dtype=np.float32)
    base = np.array([[1.0, 1.0], [1.0, -1.0]], dtype=np.float32)
    while H.shape[0] < n:
        H = np.kron(H, base)
    return H


def _build():
    nc = bacc.Bacc("TRN2", target_bir_lowering=False, debug=False)
    # x/y as [BPC, 128, 2048] bf16: same bytes as [BPC, 512, 512]; partition
    # p holds rows 4p..4p+3 (column block k of 512 = row 4p+k).
    x_d = nc.dram_tensor("x", [BPC, 128, 4 * D], BF16, kind="ExternalInput").ap()
    h128_d = nc.dram_tensor("h128", [128, 128], BF16, kind="ExternalInput").ap()
    # h1[p, k*512 + l*128 + q] = H512[4p+k, 4q+l]  (full left H, permuted)
    h1_d = nc.dram_tensor("h1", [128, 4 * N], BF16, kind="ExternalInput").ap()
    # hs[c, dt*512 + e] = H512[dt*128+c, e] / 512
    hs_d = nc.dram_tensor("hs", [128, 4 * N], BF16, kind="ExternalInput").ap()
    y_d = nc.dram_tensor("y", [BPC, 128, 4 * D], BF16, kind="ExternalOutput").ap()

    with tile.TileContext(nc) as tc:
        with (
            tc.tile_pool(name="const", bufs=1) as const_pool,
            tc.tile_pool(name="xp", bufs=4) as x_pool,
            tc.tile_pool(name="xm", bufs=3) as xm_pool,
            tc.tile_pool(name="xc", bufs=4) as xc_pool,
            tc.tile_pool(name="tp", bufs=4) as t_pool,
            tc.tile_pool(name="yp", bufs=3) as y_pool,
            tc.tile_pool(name="ps", bufs=4, space="PSUM") as psum_pool,
        ):
            W = 4 * D  # 2048, one slice's width
            xts = {}
            xcs = {}
            tts = {}
            yts = {}
            pend = []

            def emit_load(b0, butterfly=True):
                xt = x_pool.tile([128, 2 * W], BF16, name="xt")
                if butterfly:
                    nc.sync.dma_start(
                        xt[:].rearrange("p (s j) -> p s j", s=2),
                        x_d[b0 : b0 + 2].transpose([1, 0, 2]),
                    )
                else:
                    # first pair: two parallel 512 KiB DMAs so slice b0 is
                    # ready as early as possible
                    nc.sync.dma_start(xt[:, 0:W], x_d[b0])
                    nc.sync.dma_start(xt[:, W : 2 * W], x_d[b0 + 1])
                yt = y_pool.tile([128, 2 * W], BF16, name="yt")
                yts[b0] = yt
                yts[b0 + 1] = yt
                if not butterfly:
                    # first pair goes through the full-H512 left pass; the PE
                    # needs no butterfly results to start
                    xts[b0] = (xt, 0)
                    xts[b0 + 1] = (xt, W)
                    return
                # H4 combine on x: xc_l = sum_k H4[k,l] x_k
                xm = xm_pool.tile([128, 2 * W], BF16, name="xm")
                xc = xc_pool.tile([128, 2 * W], BF16, name="xc")

                def blk(t, k):
                    return t[:].rearrange("p (s j) -> p s j", s=2)[
                        :, :, k * D : (k + 1) * D
                    ]

                # level 1 all on DVE (POOL never blocks DVE)
                nc.vector.tensor_add(blk(xm, 0), blk(xt, 0), blk(xt, 1))
                nc.vector.tensor_sub(blk(xm, 1), blk(xt, 0), blk(xt, 1))
                nc.vector.tensor_add(blk(xm, 2), blk(xt, 2), blk(xt, 3))
                nc.vector.tensor_sub(blk(xm, 3), blk(xt, 2), blk(xt, 3))
                # level 2 also on DVE — GpSimd shares an SBUF port with DVE
                # (exclusive lock), so any GpSimd op doubles concurrent DVE ops
                nc.vector.tensor_add(blk(xc, 0), blk(xm, 0), blk(xm, 2))
                nc.vector.tensor_add(blk(xc, 1), blk(xm, 1), blk(xm, 3))
                nc.vector.tensor_sub(blk(xc, 2), blk(xm, 0), blk(xm, 2))
                nc.vector.tensor_sub(blk(xc, 3), blk(xm, 1), blk(xm, 3))
                xcs[b0] = (xc, 0)
                xcs[b0 + 1] = (xc, W)

            def emit_stage_a(s):
                # kron path: t_l[c, dt*128+q] = sum_p xc_l[p, dt*128+c] H128[p,q]
                # tt col layout: l*512 + dt*128 + q
                xc, xo = xcs.pop(s)
                tps = [
                    psum_pool.tile([128, 2 * N], F32, name=f"tps{h}", tag="ps")
                    for h in range(2)
                ]
                for l in range(4):
                    for dt_ in range(4):
                        nc.tensor.matmul(
                            tps[l // 2][
                                :, (l % 2) * N + dt_ * 128 : (l % 2) * N + dt_ * 128 + 128
                            ],
                            xc[:, xo + l * D + dt_ * 128 : xo + l * D + dt_ * 128 + 128],
                            h128_sb[:],
                            start=True,
                            stop=True,
                        )
                tt = t_pool.tile([128, 4 * N], BF16, name="tt")
                nc.scalar.copy(tt[:, 0 : 2 * N], tps[0][:])
                nc.scalar.copy(tt[:, 2 * N : 4 * N], tps[1][:])
                tts[s] = ("lmaj", tt)

            def emit_stage_a_full(s):
                # full-H512 left pass: tT[d, n'] with col layout dt*512+l*128+q
                xt, xo = xts.pop(s)
                tps = [
                    psum_pool.tile([128, 2 * N], F32, name=f"tps{h}", tag="ps")
                    for h in range(2)
                ]
                for dt_ in range(4):
                    out = tps[dt_ // 2][:, (dt_ % 2) * N : (dt_ % 2 + 1) * N]
                    for k in range(4):
                        nc.tensor.matmul(
                            out,
                            xt[:, xo + k * D + dt_ * 128 : xo + k * D + dt_ * 128 + 128],
                            h1_sb[:, k * N : (k + 1) * N],
                            start=(k == 0),
                            stop=(k == 3),
                        )
                tt = t_pool.tile([128, 4 * N], BF16, name="tt")
                nc.scalar.copy(tt[:, 0 : 2 * N], tps[0][:])
                nc.scalar.copy(tt[:, 2 * N : 4 * N], tps[1][:])
                tts[s] = ("dmaj", tt)

            def emit_pass2(s):
                # y[4p+k2, e] = sum_d tT[d, 4p+k2] (H512/512)[d, e]
                layout, tt = tts.pop(s)
                yt = yts.pop(s)
                yo = (s % 2) * W
                pps = [
                    psum_pool.tile([128, 2 * D], F32, name=f"pps{h}", tag="ps")
                    for h in range(2)
                ]
                for k2 in range(4):
                    for dt_ in range(4):
                        if layout == "lmaj":
                            lhsT = tt[:, k2 * N + dt_ * 128 : k2 * N + dt_ * 128 + 128]
                        else:
                            lhsT = tt[:, dt_ * N + k2 * 128 : dt_ * N + k2 * 128 + 128]
                        nc.tensor.matmul(
                            pps[k2 // 2][:, (k2 % 2) * D : (k2 % 2 + 1) * D],
                            lhsT,
                            hs_sb[:, dt_ * D : (dt_ + 1) * D],
                            start=(dt_ == 0),
                            stop=(dt_ == 3),
                        )
                    if k2 == 1:
                        nc.scalar.copy(yt[:, yo : yo + 2 * D], pps[0][:])
                nc.vector.tensor_copy(yt[:, yo + 2 * D : yo + 4 * D], pps[1][:])
                if s % 2 == 1:
                    b0 = s - 1
                    nc.sync.dma_start(
                        y_d[b0 : b0 + 2].transpose([1, 0, 2]),
                        yt[:].rearrange("p (s j) -> p s j", s=2),
                    )

            # x pair 0 and the H constants are prefetched first; slices 0-1
            # take the full-H512 left pass (no butterfly dependency)
            emit_load(0, butterfly=False)
            h1_sb = const_pool.tile([128, 4 * N], BF16, tag="h1")
            nc.sync.dma_start(h1_sb[:], h1_d[:])
            h128_sb = const_pool.tile([128, 128], BF16, tag="h128")
            nc.sync.dma_start(h128_sb[:], h128_d[:])
            hs_sb = const_pool.tile([128, 4 * N], BF16, tag="hs")
            nc.sync.dma_start(hs_sb[:], hs_d[:])

            for s in range(BPC):
                if s % 2 == 0 and s > 0:
                    emit_load(s)
                if s < 2:
                    emit_stage_a_full(s)
                else:
                    emit_stage_a(s)
                if len(pend) >= 2 or (pend and s == BPC - 1):
                    emit_pass2(pend.pop(0))
                    if s == BPC - 1 and pend:
                        emit_pass2(pend.pop(0))
                pend.append(s)
            while pend:
                emit_pass2(pend.pop(0))

    nc.compile()
    return nc


_NC = None


def kernel(x: np.ndarray) -> np.ndarray:
    global _NC, LAST_RESULTS
    if _NC is None:
        _NC = _build()
    x = (
        np.ascontiguousarray(np.asarray(x), dtype=np.float32)
        .astype(ml_dtypes.bfloat16)
        .reshape(NCORES, BPC, 128, 4 * D)
    )
    H = _hadamard(N)
    h128 = np.ascontiguousarray(_hadamard(128)).astype(ml_dtypes.bfloat16)
    h1 = np.ascontiguousarray(
        H.reshape(128, 4, 128, 4).transpose(0, 1, 3, 2).reshape(128, 4 * N)
    ).astype(ml_dtypes.bfloat16)
    hs = np.ascontiguousarray(
        H.reshape(4, 128, N).transpose(1, 0, 2).reshape(128, 4 * N)
        / np.float32(512.0)
    ).astype(ml_dtypes.bfloat16)
    in_maps = [{"x": x[i], "h128": h128, "h1": h1, "hs": hs} for i in range(NCORES)]
    trace = os.environ.get("KERNEL_TRACE", "") == "1"
    res = run_bass_kernel_spmd(_NC, in_maps, list(range(NCORES)), trace=trace)
    LAST_RESULTS = res
    out = np.stack([np.asarray(r["y"]) for r in res.results], axis=0)
    return out.reshape(B, N, D).astype(np.float32)
